# revision 7
# baseline (speedup 1.0000x reference)
"""Causal self-attention Trainium2 kernel, v3.

Full inputs in, full output out. 8 NeuronCores: data-parallel on batch (2) x
tensor-parallel on heads (4 groups of 4 heads = 2 pairs). Transposed layout
(head-dim / key-dim on partitions).

vs baseline:
- Normalization: u -> SBUF copy (DVE, f32r), f32r broadcast matmul of the raw
  denom row, reciprocal_approx_fast straight from PSUM, final multiply on
  GpSimd. No 1/8 prescale of q; exp applies scale=0.125.
- Partial projection outputs in bf16 (host sums in f32).
- Fewer DMA instructions (merged bias/mask loads, half-tile xt) for a faster
  program start.
- Global window schedule: attention chunks (pair0 0-3, pair1 1,2,3,0) with
  kqv / v-transpose / projection work as PE fillers, norm chains deferred
  one window, depth-3 S->PV pipelining.

Per-core device program (bf16 matmuls, fp32 PSUM):
  kqv^T = Wmc.T @ x^T (+bias on DVE copy-out)
  per head: S^T = k^T-block.T @ q^T-chunk   [128m x 512n], causal-trimmed
            P^T = exp(0.125 * S^T) (*mask on diag blocks)
            U^T = [v|1].T-block @ P^T       rows 0-63 sa^T raw, row 64 denom
            sa^T = U^T[0:64] * recip(bcast denom)
  partial out^T = WprojT.T @ sa^T [1024, 2048] bf16 -> DRAM
"""
import sys, os
sys.path.insert(0, '/opt/trn_rl_repo')
os.environ.setdefault("JAX_PLATFORMS", "")

import numpy as np
import ml_dtypes

import concourse.bass as bass
import concourse.bacc as bacc
import concourse.tile as tile
import concourse.mybir as mybir
from concourse import bass_utils

B, N, D, H, DH = 2, 2048, 1024, 16, 64
G = 4              # heads per core
NCORES = 8
NCH = 512          # n-chunk width
NJ = N // NCH      # 4 n-chunks
NMB = N // 128     # 16 m-blocks
bf16 = ml_dtypes.bfloat16
f32 = np.float32
AF = mybir.ActivationFunctionType

_cache = {}


def _build_program():
    nc = bacc.Bacc("TRN2", target_bir_lowering=False, debug=False, num_devices=NCORES)
    dt = mybir.dt

    xt_d = nc.dram_tensor("xt", [D, N], dt.bfloat16, kind="ExternalInput").ap()
    w_d = nc.dram_tensor("w", [6, 128, D], dt.bfloat16, kind="ExternalInput").ap()
    b_d = nc.dram_tensor("bvec", [6, 128], dt.float32, kind="ExternalInput").ap()
    wpt_d = nc.dram_tensor("wpt", [2 * 128, D], dt.bfloat16, kind="ExternalInput").ap()
    mask_d = nc.dram_tensor("masks", [4, 128, 2 * NCH], dt.bfloat16, kind="ExternalInput").ap()
    id_d = nc.dram_tensor("ident", [128, 128], dt.bfloat16, kind="ExternalInput").ap()
    ones_d = nc.dram_tensor("ones", [128, 64], dt.float32r, kind="ExternalInput").ap()
    out_d = nc.dram_tensor("outt", [D, N], dt.bfloat16, kind="ExternalOutput").ap()
    dbg = None
    if os.environ.get("KDBG") == "1":
        dbg = {
            "sa": nc.dram_tensor("dbg_sa", [2, 128, N], dt.bfloat16, kind="ExternalOutput").ap(),
            "kqvT": nc.dram_tensor("dbg_kqvT", [6, 128, N], dt.bfloat16, kind="ExternalOutput").ap(),
            "vp": nc.dram_tensor("dbg_vp", [4, 128, 16 * 66], dt.bfloat16, kind="ExternalOutput").ap(),
        }

    with tile.TileContext(nc) as tc:
        _emit(nc, tc, xt_d, w_d, b_d, wpt_d, mask_d, id_d, ones_d, out_d, dbg)

    nc.compile()
    return nc


def _emit(nc, tc, xt_d, w_d, b_d, wpt_d, mask_d, id_d, ones_d, out_d, dbg=None):
    from contextlib import ExitStack

    dt = mybir.dt
    ctx = ExitStack()
    with ctx:
        consts = ctx.enter_context(tc.tile_pool(name="consts", bufs=1))
        work = ctx.enter_context(tc.tile_pool(name="work", bufs=1))

        # ---- staged constant loads, consumption order ----
        w_sb = [None] * 6
        xt_sb = [[None] * 2 for _ in range(8)]  # per (dc, half)

        def load_w(mc):
            w_sb[mc] = consts.tile([128, D], dt.bfloat16, name=f"w{mc}", tag=f"w{mc}")
            nc.sync.dma_start(w_sb[mc][:], w_d[mc])

        xt_q = [[None] * 2 for _ in range(8)]  # quarters of half 0

        def load_xt_quarter(sub):
            for dc in range(8):
                t = consts.tile([128, NCH], dt.bfloat16,
                                name=f"xtq{dc}_{sub}", tag=f"xtq{dc}_{sub}")
                nc.sync.dma_start(
                    t[:], xt_d[dc * 128:(dc + 1) * 128,
                               sub * NCH:(sub + 1) * NCH])
                xt_q[dc][sub] = t

        def load_xt(half):
            assert half == 1
            for dc in range(8):
                t = consts.tile([128, N // 2], dt.bfloat16,
                                name=f"xt{dc}_{half}", tag=f"xt{dc}_{half}")
                nc.sync.dma_start(
                    t[:], xt_d[dc * 128:(dc + 1) * 128,
                               half * (N // 2):(half + 1) * (N // 2)])
                xt_sb[dc][half] = t

        def xt_ap(dc, jj):
            if jj < 2:
                return xt_q[dc][jj][:]
            return xt_sb[dc][1][:, (jj - 2) * NCH:(jj - 1) * NCH]

        load_w(0)
        load_w(1)
        b_sb = consts.tile([128, 6], dt.float32, name="ball", tag="ball")
        nc.sync.dma_start(b_sb[:], b_d.transpose([1, 0]))
        load_xt_quarter(0)
        load_w(2)
        load_xt_quarter(1)
        ident = consts.tile([128, 128], dt.bfloat16, name="ident", tag="ident")
        nc.sync.dma_start(ident[:], id_d[:])
        ones_sb = consts.tile([128, 64], dt.float32r, name="ones", tag="ones")
        nc.sync.dma_start(ones_sb[:], ones_d[:])
        mask_sb = consts.tile([128, 8 * NCH], dt.bfloat16, name="maskall", tag="maskall")
        nc.sync.dma_start(mask_sb[:], mask_d.transpose([1, 0, 2]))
        load_w(3)
        load_w(4)
        load_xt(1)
        load_w(5)
        wpt_sb = []
        for kc in range(2):
            t = consts.tile([128, D], dt.bfloat16, name=f"wpt{kc}", tag=f"wpt{kc}")
            nc.sync.dma_start(t[:], wpt_d[kc * 128:(kc + 1) * 128, :])
            wpt_sb.append(t)

        def mask_ap(r):
            return mask_sb[:, r * 2 * NCH:(r + 1) * 2 * NCH]

        # ---- persistent work tiles ----
        kqvT = [work.tile([128, N], dt.bfloat16, name=f"kqvT{i}", tag=f"kqvT{i}")
                for i in range(6)]
        vp = [work.tile([128, NMB, 66], dt.bfloat16, name=f"vp{h}", tag=f"vp{h}")
              for h in range(G)]
        saT = [work.tile([128, N], dt.bfloat16, name=f"saT{kc}", tag=f"saT{kc}")
               for kc in range(2)]
        for h in range(G):
            nc.gpsimd.memset(vp[h][:, :, 64:65], 1.0)

        # ---- pools ----
        ps = ctx.enter_context(tc.tile_pool(name="ps", bufs=2, space="PSUM"))
        pu = ctx.enter_context(tc.tile_pool(name="pu", bufs=2, space="PSUM"))
        pp = ctx.enter_context(tc.tile_pool(name="pp", bufs=2, space="PSUM"))
        pPool = ctx.enter_context(tc.tile_pool(name="pP", bufs=9))
        pun = ctx.enter_context(tc.tile_pool(name="pun", bufs=5))
        paux = ctx.enter_context(tc.tile_pool(name="paux", bufs=6))
        pout = ctx.enter_context(tc.tile_pool(name="pout", bufs=6))

        def emit_kqv_group(p, sec, jj):
            """sec: 0=k, 1=q, 2=v. Copy-out applies bias."""
            mc = 3 * p + sec
            ps_t = pp.tile([128, NCH], dt.float32, tag="pp", name="kqvp")
            for dc in range(8):
                nc.tensor.matmul(
                    ps_t[:],
                    w_sb[mc][:, dc * 128:(dc + 1) * 128],
                    xt_ap(dc, jj),
                    start=(dc == 0), stop=(dc == 7),
                )
            nc.vector.tensor_scalar_add(
                kqvT[mc][:, jj * NCH:(jj + 1) * NCH], ps_t[:], b_sb[:, mc:mc + 1])

        def head_slices(h):
            p, o = h // 2, (h % 2) * 64
            kT = kqvT[3 * p][o:o + 64, :]
            qT = kqvT[3 * p + 1][o:o + 64, :]
            vT = kqvT[3 * p + 2][o:o + 64, :]
            return kT, qT, vT, o

        def emit_vp_quad(h, q):
            _, _, vT_h, o = head_slices(h)
            tp = pp.tile([128, 256], dt.bfloat16, tag="pp", name="vtp")
            for i in range(4):
                mb = 4 * q + i
                nc.tensor.matmul(
                    tp[:, 64 * i:64 * (i + 1)],
                    vT_h[:, mb * 128:(mb + 1) * 128],
                    ident[o:o + 64, o:o + 64],
                    is_transpose=True, skip_group_check=True,
                )
            nc.vector.tensor_copy(vp[h][:, 4 * q:4 * q + 4, 0:64], tp[:])

        def emit_proj_oc(j, oc, scalar_copy=False):
            nsl = slice(j * NCH, (j + 1) * NCH)
            pp_t = pp.tile([128, NCH], dt.float32, tag="pp", name="pp_t")
            for kc in range(2):
                nc.tensor.matmul(
                    pp_t[:],
                    wpt_sb[kc][:, oc * 128:(oc + 1) * 128],
                    saT[kc][:, nsl],
                    start=(kc == 0), stop=(kc == 1),
                )
            o_t = pout.tile([128, NCH], dt.bfloat16, tag="o", name="o_t")
            if scalar_copy:
                nc.scalar.copy(o_t[:], pp_t[:])
            else:
                nc.vector.tensor_copy(o_t[:], pp_t[:])
            nc.sync.dma_start(out_d[oc * 128:(oc + 1) * 128, nsl], o_t[:])

        def norm_steps(h, j, u_t):
            kc, row = h // 2, (h % 2) * 64
            nsl = slice(j * NCH, (j + 1) * NCH)
            st = {}

            def s1():
                st["u_sb"] = pun.tile([65, NCH], dt.float32r, tag="un", name="usb")
                nc.vector.tensor_copy(st["u_sb"][0:65, :], u_t[0:65, :])

            def s2():
                bt = ps.tile([128, 2 * NCH], dt.float32, tag="s2", name="bcp")
                st["bcp"] = bt[0:64, 0:NCH]
                nc.tensor.matmul(
                    st["bcp"],
                    ones_sb[64:65, 0:64],
                    st["u_sb"][64:65, :],
                    start=True, stop=True,
                )

            def s3():
                st["rc"] = paux.tile([64, NCH], dt.float32, tag="rc", name="rc")
                nc.vector.reciprocal_approx_fast(st["rc"][:], st["bcp"])

            def s4():
                u_f32 = st["u_sb"][0:64, :].bitcast(dt.float32)
                if row == 0:
                    nc.gpsimd.tensor_mul(saT[kc][0:64, nsl], u_f32, st["rc"][:])
                else:
                    tmp = paux.tile([64, NCH], dt.bfloat16, tag="tmp", name="tmp")
                    nc.gpsimd.tensor_mul(tmp[:], u_f32, st["rc"][:])
                    nc.sync.dma_start(saT[kc][64:128, nsl], tmp[:])

            return [s1, s2, s3, s4]

        def emit_attn_chunk(j, p, fillers, norm_prev, gated=()):
            """Attention chunk j for pair p. `fillers`: dependency-free PE work.
            `norm_prev`: deferred norm steps of the previous chunk. `gated`:
            fillers that must be EMITTED only after all norm_prev steps (they
            read tiles norm_prev writes)."""
            nm = 4 * (j + 1)
            pair = (2 * p, 2 * p + 1)
            u_t = {h: pu.tile([65, NCH], dt.float32, tag="u", name=f"u{h}")
                   for h in pair}
            p_tiles = {h: [None] * nm for h in pair}
            offs = [0] * nm
            # interleave norm steps with fills (keeps each norm step's
            # producers a few PE ops ahead of its consumers), gated work last
            from itertools import zip_longest
            units = [u for pair_ in zip_longest(list(norm_prev), list(fillers))
                     for u in pair_ if u is not None]
            units += list(gated)
            total = len(units)
            acc = [0]

            def drain_share():
                # Bresenham: spread `total` units evenly over nm m-steps
                acc[0] += total
                while acc[0] >= nm and units:
                    acc[0] -= nm
                    units.pop(0)()

            def pair_view(t, off):
                return t[:, :].rearrange("p (i n) -> p i n", i=2)[:, :, off:]

            def emit_s_pair(mi):
                r = mi - 4 * j
                off = 128 * r if r > 0 else 0
                offs[mi] = off
                s2t = ps.tile([128, 2 * NCH], dt.float32, tag="s2", name="s2t")
                for idx, h in enumerate(pair):
                    kT, qT, _, _ = head_slices(h)
                    nc.tensor.matmul(
                        s2t[:, idx * NCH + off:(idx + 1) * NCH],
                        kT[:, mi * 128:(mi + 1) * 128],
                        qT[:, j * NCH + off:(j + 1) * NCH],
                        start=True, stop=True, skip_group_check=True,
                    )
                p_t = pPool.tile([128, 2 * NCH], dt.bfloat16, tag="p", name="p_t")
                if r >= 0:
                    e_t = pPool.tile([128, 2 * NCH], dt.bfloat16, tag="e", name="e_t")
                    nc.scalar.activation(pair_view(e_t, off), pair_view(s2t, off),
                                         AF.Exp, scale=0.125)
                    nc.vector.tensor_mul(
                        pair_view(p_t, off), pair_view(e_t, off),
                        mask_ap(r).rearrange("p (i n) -> p i n", i=2)[:, :, off:])
                else:
                    nc.scalar.activation(pair_view(p_t, off), pair_view(s2t, off),
                                         AF.Exp, scale=0.125)
                for h in pair:
                    p_tiles[h][mi] = p_t

            def emit_pv(h, mi):
                off = offs[mi]
                idx = h % 2
                nc.tensor.matmul(
                    u_t[h][:, off:],
                    vp[h][:, mi, 0:65],
                    p_tiles[h][mi][:, idx * NCH + off:(idx + 1) * NCH],
                    start=(mi == 0), stop=(mi == nm - 1),
                    skip_group_check=True,
                )

            depth = 4
            for mi in range(nm):
                drain_share()
                emit_s_pair(mi)
                if mi >= depth:
                    for h in pair:
                        emit_pv(h, mi - depth)
            for mi in range(max(nm - depth, 0), nm):
                for h in pair:
                    emit_pv(h, mi)
            while units:
                units.pop(0)()

            return [st for h in pair for st in norm_steps(h, j, u_t[h])]

        # ---- window schedule ----
        KQ = emit_kqv_group
        VQ = emit_vp_quad

        def F(fn, *a):
            return lambda: fn(*a)

        # A-mini: enough for attn (p0, 0)
        KQ(0, 0, 0); KQ(0, 1, 0); KQ(0, 2, 0)
        VQ(0, 0); VQ(1, 0)

        windows = [
            (0, 0, [F(KQ, 0, 0, 1), F(KQ, 0, 1, 1)], []),
            (0, 1, [F(KQ, 0, 2, 1), F(VQ, 0, 1), F(VQ, 1, 1),
                    F(KQ, 0, 0, 2), F(KQ, 0, 1, 2)], []),
            (0, 2, [F(KQ, 0, 2, 2), F(VQ, 0, 2), F(VQ, 1, 2),
                    F(KQ, 0, 0, 3), F(KQ, 0, 1, 3),
                    F(KQ, 1, 0, 0), F(KQ, 1, 1, 0)], []),
            (0, 3, [F(KQ, 0, 2, 3), F(VQ, 0, 3), F(VQ, 1, 3),
                    F(KQ, 1, 2, 0), F(KQ, 1, 0, 1), F(KQ, 1, 1, 1),
                    F(KQ, 1, 2, 1), F(KQ, 1, 0, 2)], []),
            (1, 1, [F(VQ, 2, 0), F(VQ, 3, 0), F(VQ, 2, 1), F(VQ, 3, 1),
                    F(KQ, 1, 1, 2), F(KQ, 1, 2, 2),
                    F(KQ, 1, 0, 3), F(KQ, 1, 1, 3)], []),
            (1, 2, [F(KQ, 1, 2, 3), F(VQ, 2, 2), F(VQ, 3, 2)],
                   [F(emit_proj_oc, 1, oc) for oc in range(2)]),
            (1, 3, [F(VQ, 2, 3), F(VQ, 3, 3)]
                   + [F(emit_proj_oc, 1, oc) for oc in range(2, 8)],
                   [F(emit_proj_oc, 2, oc) for oc in range(6)]),
            (1, 0, [F(emit_proj_oc, 2, oc) for oc in range(6, 8)],
                   [F(emit_proj_oc, 3, oc, oc % 2 == 1) for oc in range(8)]),
        ]

        pending_norm = []
        for p, j, fills, gated in windows:
            pending_norm = emit_attn_chunk(j, p, fills, pending_norm, gated)
        # tail: last chunk's norm (e/o interleaved) + its projection
        e_st, o_st = pending_norm[0:4], pending_norm[4:8]
        for a, b in zip(e_st, o_st):
            a(); b()
        for oc in range(8):
            emit_proj_oc(0, oc, scalar_copy=(oc % 2 == 1))

        if dbg is not None:
            for kc in range(2):
                nc.sync.dma_start(dbg["sa"][kc], saT[kc][:])
            for i in range(6):
                nc.sync.dma_start(dbg["kqvT"][i], kqvT[i][:])
            for h in range(4):
                nc.sync.dma_start(dbg["vp"][h], vp[h].rearrange("p a b -> p (a b)"))


def _host_prep(x, W_kqv, b_kqv, W_proj):
    """Build the 8 per-core input maps."""
    x = np.asarray(x, dtype=f32)
    W_kqv = np.asarray(W_kqv, dtype=f32)
    b_kqv = np.asarray(b_kqv, dtype=f32)
    W_proj = np.asarray(W_proj, dtype=f32)

    masks = np.zeros((4, 128, 2 * NCH), dtype=bf16)
    mm = np.arange(128)[:, None]
    nn = np.arange(NCH)[None, :]
    for r in range(4):
        m1 = (nn >= mm + 128 * r).astype(bf16)
        masks[r] = np.concatenate([m1, m1], axis=1)
    ident = np.eye(128, dtype=bf16)

    in_maps = []
    for c in range(NCORES):
        b, g = c // 4, c % 4
        heads = [4 * g + i for i in range(4)]
        wtiles, btiles = [], []
        for p in range(2):
            he, ho = heads[2 * p], heads[2 * p + 1]
            for sec in range(3):  # k, q, v
                blk = np.concatenate(
                    [W_kqv[h][:, sec * 64:(sec + 1) * 64] for h in (he, ho)], axis=1)
                wtiles.append(blk.reshape(8, 128, 128).transpose(1, 0, 2)
                              .reshape(128, 1024).astype(bf16))
                btiles.append(np.concatenate(
                    [b_kqv[h][sec * 64:(sec + 1) * 64] for h in (he, ho)]
                ).astype(f32))
        in_maps.append({
            "xt": np.ascontiguousarray(x[b].T).astype(bf16),
            "w": np.stack(wtiles),
            "bvec": np.stack(btiles),
            "wpt": np.ascontiguousarray(W_proj[:, 256 * g:256 * (g + 1)].T).astype(bf16),
            "masks": masks,
            "ident": ident,
            "ones": np.ones((128, 64), dtype=f32),
        })
    return in_maps


def run(x, W_kqv, b_kqv, W_proj, b_proj, trace=False, trace_cores=None):
    if "nc" not in _cache:
        _cache["nc"] = _build_program()
    nc = _cache["nc"]
    in_maps = _host_prep(x, W_kqv, b_kqv, W_proj)
    res = bass_utils.run_bass_kernel_spmd(
        nc, in_maps, core_ids=list(range(NCORES)),
        trace=trace, trace_cores=trace_cores,
    )
    b_proj = np.asarray(b_proj, dtype=f32)
    out = np.zeros((B, N, D), dtype=f32)
    for b in range(B):
        acc = res.results[4 * b]["outt"].astype(f32)
        for g in range(1, 4):
            acc = acc + res.results[4 * b + g]["outt"].astype(f32)
        out[b] = acc.T + b_proj[None, :]
    return out, res


def kernel(x, W_kqv, b_kqv, W_proj, b_proj):
    out, _ = run(x, W_kqv, b_kqv, W_proj, b_proj, trace=False)
    return out
